# revision 8
# baseline (speedup 1.0000x reference)
# Trainium2 Bass kernel for per-sample channel-attention module (CAM).
#
# Reference math per sample (C=512, N=H*W=4096):
#   X = x.reshape(C, N)
#   phi = Wp X ; theta = Wt X ; g = Wg X
#   attn = softmax_rows(phi @ theta^T)          # [C, C]
#   y = attn @ g                                 # [C, N]
#   Z = (y^T).flatten().reshape(C, N)            # torch permute+view reinterpretation
#   out = gamma * (Wm @ Z) + x
#
# Algebraic restructuring (cuts PE work ~1.8x vs the naive 6-GEMM chain):
#   G = X X^T                  (Gram, [C, C])
#   L = Wp G Wt^T              (attention logits via two small GEMMs)
#   A' = softmax(L) @ Wg       (fold g-projection into attention)
#   y = A' X                   (single big GEMM)
# The torch permute+view reinterpretation is free: y^T blocks are produced
# with a stride-8 column selection of X as the stationary matmul operand, so
# each PSUM tile lands exactly on a contiguous block of Z's SBUF layout.
#
# All matmuls run in float32r (fp32 data streamed through the PE at
# 1 row/cycle; operands carry 11 explicit mantissa bits, RNE). The host
# pre-rounds inputs to the fp32r grid and also pre-computes pure layout
# transforms: X^T, Wp^T, Wt^T, and gamma*Wm^T (zero FLOPs of the reference
# are moved off-device; every GEMM/softmax runs on the NeuronCore).

import os
import numpy as np

import concourse.bass as bass
import concourse.mybir as mybir
import concourse.tile as tile
from concourse import bacc
from concourse.bass_utils import run_bass_kernel_spmd
from concourse.tile import TileContext
from concourse.masks import make_identity

P = 128          # partitions
C = 512          # channels
N = 4096         # spatial (64*64)
CC = C // P      # 4 channel chunks
NT = N // P      # 32 spatial tiles
QF = N // C      # 8 fold factor for the permute+view reinterpretation
FP32 = mybir.dt.float32
FP32R = mybir.dt.float32r


def _f32(ap):
    # reinterpret an fp32r tile as plain fp32 (identical bit layout)
    return ap.bitcast(FP32)


def _build_nc():
    nc = bacc.Bacc("TRN2", target_bir_lowering=False, debug=False, num_devices=8)
    x_d = nc.dram_tensor("x", [C, N], FP32R, kind="ExternalInput").ap()
    xt_d = nc.dram_tensor("xt", [N, C], FP32R, kind="ExternalInput").ap()
    wphiT_d = nc.dram_tensor("w_phi_t", [C, C], FP32R, kind="ExternalInput").ap()
    wthetaT_d = nc.dram_tensor("w_theta_t", [C, C], FP32R, kind="ExternalInput").ap()
    wg_d = nc.dram_tensor("w_g", [C, C], FP32R, kind="ExternalInput").ap()
    wmTg_d = nc.dram_tensor("w_mask_t_g", [C, C], FP32R, kind="ExternalInput").ap()
    out_d = nc.dram_tensor("out", [C, N], FP32, kind="ExternalOutput").ap()

    with TileContext(nc) as tc:
        _body(tc, x_d, xt_d, wphiT_d, wthetaT_d, wg_d, wmTg_d, out_d)
    nc.compile()
    return nc


def _body(tc, x_d, xt_d, wphiT_d, wthetaT_d, wg_d, wmTg_d, out_d):
    nc = tc.nc
    from contextlib import ExitStack

    with ExitStack() as ctx:
        const = ctx.enter_context(tc.tile_pool(name="const", bufs=1))
        xpool = ctx.enter_context(tc.tile_pool(name="xpool", bufs=1))
        wpool = ctx.enter_context(tc.tile_pool(name="wpool", bufs=1))
        bigpool = ctx.enter_context(tc.tile_pool(name="bigpool", bufs=1))
        scratch = ctx.enter_context(tc.tile_pool(name="scratch", bufs=2))
        vecs = ctx.enter_context(tc.tile_pool(name="vecs", bufs=8))
        outp = ctx.enter_context(tc.tile_pool(name="outp", bufs=4))
        ps = ctx.enter_context(tc.tile_pool(name="ps", bufs=4, space="PSUM"))
        psg = ctx.enter_context(tc.tile_pool(name="psg", bufs=4, space="PSUM"))

        identity = const.tile([P, P], FP32)
        make_identity(nc, identity)

        # ---- weight loads on the SWDGE path (gpsimd) so they never delay the
        # latency-critical xt stream on the Sync HWDGE queue.
        # Layout [p, cc, j]: tile[p, cc, j] = W[128*cc + p, j].
        wphiT = wpool.tile([P, CC, C], FP32R)
        wthetaT = wpool.tile([P, CC, C], FP32R)
        wg_sb = wpool.tile([P, CC, C], FP32R)
        wmT = wpool.tile([P, CC, C], FP32R)
        for w_d, wt in (
            (wphiT_d, wphiT),
            (wthetaT_d, wthetaT),
            (wg_d, wg_sb),
            (wmTg_d, wmT),
        ):
            nc.gpsimd.dma_start(
                out=wt, in_=w_d.rearrange("(cc p) j -> p cc j", p=P)
            )

        # x in natural layout, needed from the ZS phase on (not urgent).
        x_sb = xpool.tile([P, CC, N], FP32R)
        for cc in range(CC):
            nc.gpsimd.dma_start(
                out=x_sb[:, cc, :], in_=x_d[cc * P:(cc + 1) * P, :]
            )

        # ---- stream X^T tiles and fold each into the Gram accumulators.
        # XT[p, t, c] = X[c, 128*t + p];  G[a, b] = sum_n X[a, n] X[b, n].
        xt_sb = bigpool.tile([P, NT, C], FP32R, tag="big")
        gacc = [
            psg.tile([P, C], FP32, tag="gacc", name=f"gacc{i}")
            for i in range(CC)
        ]
        for t in range(NT):
            nc.sync.dma_start(
                out=xt_sb[:, t, :], in_=xt_d[t * P:(t + 1) * P, :]
            )
            for mc in range(CC):
                nc.tensor.matmul(
                    gacc[mc],
                    xt_sb[:, t, mc * P:(mc + 1) * P],
                    xt_sb[:, t, :],
                    start=(t == 0),
                    stop=(t == NT - 1),
                )

        g_sb = scratch.tile([P, CC, C], FP32R, tag="s8")
        for mc in range(CC):
            nc.any.tensor_copy(g_sb[:, mc, :], gacc[mc])

        # ---- T1 = G @ Wt^T  (uses G symmetry for the stationary operand)
        t1_sb = scratch.tile([P, CC, C], FP32R, tag="s8")
        for mc in range(CC):
            tp = ps.tile([P, C], FP32, tag="ps")
            for jc in range(CC):
                nc.tensor.matmul(
                    tp,
                    g_sb[:, jc, mc * P:(mc + 1) * P],
                    wthetaT[:, jc, :],
                    start=(jc == 0),
                    stop=(jc == CC - 1),
                )
            nc.any.tensor_copy(t1_sb[:, mc, :], tp)

        # ---- L = Wp @ T1 ; softmax rows -> attn
        attn_sb = scratch.tile([P, CC, C], FP32R, tag="s8")
        for mc in range(CC):
            lp = ps.tile([P, C], FP32, tag="ps")
            for ic in range(CC):
                nc.tensor.matmul(
                    lp,
                    wphiT[:, ic, mc * P:(mc + 1) * P],
                    t1_sb[:, ic, :],
                    start=(ic == 0),
                    stop=(ic == CC - 1),
                )
            neg_max = vecs.tile([P, 1], FP32)
            nc.vector.tensor_reduce(
                out=neg_max, in_=lp, axis=mybir.AxisListType.X,
                op=mybir.AluOpType.max, negate=True,
            )
            sums = vecs.tile([P, 1], FP32)
            nc.scalar.activation(
                out=attn_sb[:, mc, :], in_=lp,
                func=mybir.ActivationFunctionType.Exp,
                bias=neg_max, scale=1.0, accum_out=sums,
            )
            rinv = vecs.tile([P, 1], FP32)
            nc.vector.reciprocal(rinv, sums)
            nc.vector.tensor_scalar_mul(
                attn_sb[:, mc, :], attn_sb[:, mc, :], rinv
            )

        # ---- attn^T via PE transposes (fp32 mode; copies round to fp32r)
        attnT_sb = scratch.tile([P, CC, C], FP32R, tag="s8")
        for dc in range(CC):
            pt = ps.tile([P, C], FP32, tag="ps")
            for mc in range(CC):
                nc.tensor.transpose(
                    pt[:, mc * P:(mc + 1) * P],
                    _f32(attn_sb[:, mc, dc * P:(dc + 1) * P]),
                    identity,
                )
            nc.any.tensor_copy(attnT_sb[:, dc, :], pt)

        # ---- A'^T[j, c] = sum_d Wg[d, j] attn[c, d]
        apT_sb = scratch.tile([P, CC, C], FP32R, tag="s8")
        for jc in range(CC):
            ap_ps = ps.tile([P, C], FP32, tag="ps")
            for dc in range(CC):
                nc.tensor.matmul(
                    ap_ps,
                    wg_sb[:, dc, jc * P:(jc + 1) * P],
                    attnT_sb[:, dc, :],
                    start=(dc == 0),
                    stop=(dc == CC - 1),
                )
            nc.any.tensor_copy(apT_sb[:, jc, :], ap_ps)

        # ---- y^T blocks straight into Z layout, interleaved q-major with the
        # final mask GEMM + residual + store.
        # Z[i, q*512 + r] = y^T[8*i + q, r]; with n = 1024*ci + 8*m + q the
        # output PSUM tile [m, r] equals ZS[:, ci, q*512:(q+1)*512], and the
        # mask GEMM for output block jb=q only needs ZS blocks (ci=0..3, q).
        zs_sb = bigpool.tile([P, CC, N], FP32R, tag="big")
        for q in range(QF):
            for ci in range(CC):
                zp = ps.tile([P, C], FP32, tag="ps")
                for jc in range(CC):
                    xr = x_sb[:, jc, :].rearrange(
                        "p (ci m q) -> p ci q m", ci=CC, q=QF
                    )
                    nc.tensor.matmul(
                        zp,
                        xr[:, ci, q, :],
                        apT_sb[:, jc, :],
                        start=(jc == 0),
                        stop=(jc == CC - 1),
                    )
                nc.any.tensor_copy(zs_sb[:, ci, q * C:(q + 1) * C], zp)

            jb = q
            for oc in range(CC):
                mp = psg.tile([P, C], FP32, tag="gacc")
                for ic in range(CC):
                    nc.tensor.matmul(
                        mp,
                        wmT[:, ic, oc * P:(oc + 1) * P],
                        zs_sb[:, ic, jb * C:(jb + 1) * C],
                        start=(ic == 0),
                        stop=(ic == CC - 1),
                    )
                ot = outp.tile([P, C], FP32)
                nc.vector.tensor_add(
                    ot, mp, _f32(x_sb[:, oc, jb * C:(jb + 1) * C])
                )
                nc.sync.dma_start(
                    out=out_d[oc * P:(oc + 1) * P, jb * C:(jb + 1) * C], in_=ot
                )


_NC_CACHE = {}
LAST_RESULT = None


def get_nc():
    if "nc" not in _NC_CACHE:
        _NC_CACHE["nc"] = _build_nc()
    return _NC_CACHE["nc"]


def _round_fp32r(x):
    """Round fp32 array to the fp32r grid (11 explicit mantissa bits, RNE)."""
    u = np.ascontiguousarray(x, dtype=np.float32).view(np.uint32).astype(np.uint64)
    shift = 23 - 11
    add = (np.uint64(1) << np.uint64(shift - 1)) - np.uint64(1) + (
        (u >> np.uint64(shift)) & np.uint64(1)
    )
    u = (u + add) & np.uint64(~((1 << shift) - 1) & 0xFFFFFFFF)
    return u.astype(np.uint32).view(np.float32)


def make_in_map(xb, w_phi_t, w_theta_t, w_g, w_mask_t_g):
    """Per-core input dict; xb is one sample [C, H, W]."""
    xr = _round_fp32r(xb.reshape(C, N))
    return {
        "x": xr,
        "xt": np.ascontiguousarray(xr.T),
        "w_phi_t": w_phi_t,
        "w_theta_t": w_theta_t,
        "w_g": w_g,
        "w_mask_t_g": w_mask_t_g,
    }


def prep_weights(w_phi, w_theta, w_g, w_mask, gamma):
    w_phi_t = _round_fp32r(np.asarray(w_phi, dtype=np.float32).T)
    w_theta_t = _round_fp32r(np.asarray(w_theta, dtype=np.float32).T)
    w_g_r = _round_fp32r(np.asarray(w_g, dtype=np.float32))
    gamma64 = float(np.asarray(gamma, dtype=np.float32).reshape(-1)[0])
    w_mask_t_g = _round_fp32r(
        (np.asarray(w_mask, dtype=np.float64).T * gamma64).astype(np.float32)
    )
    return w_phi_t, w_theta_t, w_g_r, w_mask_t_g


def kernel(x, w_phi, w_theta, w_g, w_mask, gamma):
    global LAST_RESULT
    x = np.ascontiguousarray(np.asarray(x, dtype=np.float32))
    B, c, h, w = x.shape
    assert (c, h * w) == (C, N), (x.shape,)

    w_phi_t, w_theta_t, w_g_r, w_mask_t_g = prep_weights(
        w_phi, w_theta, w_g, w_mask, gamma
    )
    nc = get_nc()
    in_maps = [
        make_in_map(x[b], w_phi_t, w_theta_t, w_g_r, w_mask_t_g)
        for b in range(B)
    ]
    trace = bool(int(os.environ.get("KERNEL_TRACE", "0")))
    res = run_bass_kernel_spmd(nc, in_maps, list(range(B)), trace=trace)
    LAST_RESULT = res
    out = np.stack([res.results[b]["out"].reshape(c, h, w) for b in range(B)])
    return out


# revision 9
# speedup vs baseline: 1.1219x; 1.1219x over previous
# Trainium2 Bass kernel for per-sample channel-attention module (CAM).
#
# Reference math per sample (C=512, N=H*W=4096):
#   X = x.reshape(C, N)
#   phi = Wp X ; theta = Wt X ; g = Wg X
#   attn = softmax_rows(phi @ theta^T)          # [C, C]
#   y = attn @ g                                 # [C, N]
#   Z = (y^T).flatten().reshape(C, N)            # torch permute+view reinterpretation
#   out = gamma * (Wm @ Z) + x
#
# Algebraic restructuring (cuts PE work ~1.8x vs the naive 6-GEMM chain):
#   G = X X^T                  (Gram, [C, C])
#   L = Wp G Wt^T              (attention logits via two small GEMMs)
#   A' = softmax(L) @ Wg       (fold g-projection into attention)
#   y = A' X                   (single big GEMM)
# The torch permute+view reinterpretation is free: y^T blocks are produced
# with a stride-8 column selection of X as the stationary matmul operand, so
# each PSUM tile lands exactly on a contiguous block of Z's SBUF layout.
#
# All matmuls run in float32r (fp32 data streamed through the PE at
# 1 row/cycle; operands carry 11 explicit mantissa bits, RNE). The host
# pre-rounds inputs to the fp32r grid and also pre-computes pure layout
# transforms: X^T, Wp^T, Wt^T, and gamma*Wm^T (zero FLOPs of the reference
# are moved off-device; every GEMM/softmax runs on the NeuronCore).

import os
import numpy as np

import concourse.bass as bass
import concourse.mybir as mybir
import concourse.tile as tile
from concourse import bacc
from concourse.bass_utils import run_bass_kernel_spmd
from concourse.tile import TileContext
from concourse.masks import make_identity

P = 128          # partitions
C = 512          # channels
N = 4096         # spatial (64*64)
CC = C // P      # 4 channel chunks
NT = N // P      # 32 spatial tiles
QF = N // C      # 8 fold factor for the permute+view reinterpretation
FP32 = mybir.dt.float32
FP32R = mybir.dt.float32r


def _f32(ap):
    # reinterpret an fp32r tile as plain fp32 (identical bit layout)
    return ap.bitcast(FP32)


def _build_nc():
    nc = bacc.Bacc("TRN2", target_bir_lowering=False, debug=False, num_devices=8)
    x_d = nc.dram_tensor("x", [C, N], FP32R, kind="ExternalInput").ap()
    xt_d = nc.dram_tensor("xt", [N, C], FP32R, kind="ExternalInput").ap()
    wphiT_d = nc.dram_tensor("w_phi_t", [C, C], FP32R, kind="ExternalInput").ap()
    wthetaT_d = nc.dram_tensor("w_theta_t", [C, C], FP32R, kind="ExternalInput").ap()
    wg_d = nc.dram_tensor("w_g", [C, C], FP32R, kind="ExternalInput").ap()
    wmTg_d = nc.dram_tensor("w_mask_t_g", [C, C], FP32R, kind="ExternalInput").ap()
    out_d = nc.dram_tensor("out", [C, N], FP32, kind="ExternalOutput").ap()

    with TileContext(nc) as tc:
        _body(tc, x_d, xt_d, wphiT_d, wthetaT_d, wg_d, wmTg_d, out_d)
    nc.compile()
    return nc


def _body(tc, x_d, xt_d, wphiT_d, wthetaT_d, wg_d, wmTg_d, out_d):
    nc = tc.nc
    from contextlib import ExitStack

    with ExitStack() as ctx:
        const = ctx.enter_context(tc.tile_pool(name="const", bufs=1))
        xpool = ctx.enter_context(tc.tile_pool(name="xpool", bufs=1))
        wpool = ctx.enter_context(tc.tile_pool(name="wpool", bufs=1))
        bigpool = ctx.enter_context(tc.tile_pool(name="bigpool", bufs=1))
        scratch = ctx.enter_context(tc.tile_pool(name="scratch", bufs=2))
        vecs = ctx.enter_context(tc.tile_pool(name="vecs", bufs=8))
        outp = ctx.enter_context(tc.tile_pool(name="outp", bufs=4))
        ps = ctx.enter_context(tc.tile_pool(name="ps", bufs=4, space="PSUM"))
        psg = ctx.enter_context(tc.tile_pool(name="psg", bufs=4, space="PSUM"))

        identity = const.tile([P, P], FP32)
        make_identity(nc, identity)

        # ---- weight loads on the SWDGE path (gpsimd) so they never delay the
        # latency-critical xt stream on the Sync HWDGE queue.
        # Layout [p, cc, j]: tile[p, cc, j] = W[128*cc + p, j].
        wphiT = wpool.tile([P, CC, C], FP32R)
        wthetaT = wpool.tile([P, CC, C], FP32R)
        wg_sb = wpool.tile([P, CC, C], FP32R)
        wmT = wpool.tile([P, CC, C], FP32R)
        # Weight + x loads go on the ACT HWDGE queue (nc.scalar.dma_start)
        # so the latency-critical xt stream owns the Sync queue. Order by
        # first use: wthetaT (T1), wphiT (L), wg (A'), then x, then wmT.
        x_sb = xpool.tile([P, CC, N], FP32R)
        for w_d, wt in (
            (wthetaT_d, wthetaT),
            (wphiT_d, wphiT),
            (wg_d, wg_sb),
        ):
            nc.scalar.dma_start(
                out=wt, in_=w_d.rearrange("(cc p) j -> p cc j", p=P)
            )
        for cc in range(CC):
            nc.scalar.dma_start(
                out=x_sb[:, cc, :], in_=x_d[cc * P:(cc + 1) * P, :]
            )
        nc.scalar.dma_start(
            out=wmT, in_=wmTg_d.rearrange("(cc p) j -> p cc j", p=P)
        )

        # ---- stream X^T in 1 MB chunks (4 tiles each) and fold each tile
        # into the Gram accumulators as soon as its chunk lands.
        # XT[p, t, c] = X[c, 128*t + p];  G[a, b] = sum_n X[a, n] X[b, n].
        xt_sb = bigpool.tile([P, NT, C], FP32R, tag="big")
        gacc = [
            psg.tile([P, C], FP32, tag="gacc", name=f"gacc{i}")
            for i in range(CC)
        ]
        for tg in range(NT // 4):
            nc.sync.dma_start(
                out=xt_sb[:, tg * 4:(tg + 1) * 4, :],
                in_=xt_d[tg * 4 * P:(tg + 1) * 4 * P, :].rearrange(
                    "(tt p) c -> p tt c", p=P
                ),
            )
            for k in range(4):
                t = tg * 4 + k
                for mc in range(CC):
                    nc.tensor.matmul(
                        gacc[mc],
                        xt_sb[:, t, mc * P:(mc + 1) * P],
                        xt_sb[:, t, :],
                        start=(t == 0),
                        stop=(t == NT - 1),
                    )

        g_sb = scratch.tile([P, CC, C], FP32R, tag="s8")
        for mc in range(CC):
            nc.any.tensor_copy(g_sb[:, mc, :], gacc[mc])

        # ---- T1 = G @ Wt^T  (uses G symmetry for the stationary operand)
        t1_sb = scratch.tile([P, CC, C], FP32R, tag="s8")
        for mc in range(CC):
            tp = ps.tile([P, C], FP32, tag="ps")
            for jc in range(CC):
                nc.tensor.matmul(
                    tp,
                    g_sb[:, jc, mc * P:(mc + 1) * P],
                    wthetaT[:, jc, :],
                    start=(jc == 0),
                    stop=(jc == CC - 1),
                )
            nc.any.tensor_copy(t1_sb[:, mc, :], tp)

        # ---- L = Wp @ T1 ; softmax rows -> attn
        attn_sb = scratch.tile([P, CC, C], FP32R, tag="s8")
        for mc in range(CC):
            lp = ps.tile([P, C], FP32, tag="ps")
            for ic in range(CC):
                nc.tensor.matmul(
                    lp,
                    wphiT[:, ic, mc * P:(mc + 1) * P],
                    t1_sb[:, ic, :],
                    start=(ic == 0),
                    stop=(ic == CC - 1),
                )
            neg_max = vecs.tile([P, 1], FP32)
            nc.vector.tensor_reduce(
                out=neg_max, in_=lp, axis=mybir.AxisListType.X,
                op=mybir.AluOpType.max, negate=True,
            )
            sums = vecs.tile([P, 1], FP32)
            nc.scalar.activation(
                out=attn_sb[:, mc, :], in_=lp,
                func=mybir.ActivationFunctionType.Exp,
                bias=neg_max, scale=1.0, accum_out=sums,
            )
            rinv = vecs.tile([P, 1], FP32)
            nc.vector.reciprocal(rinv, sums)
            nc.vector.tensor_scalar_mul(
                attn_sb[:, mc, :], attn_sb[:, mc, :], rinv
            )

        # ---- attn^T via PE transposes (fp32 mode; copies round to fp32r)
        attnT_sb = scratch.tile([P, CC, C], FP32R, tag="s8")
        for dc in range(CC):
            pt = ps.tile([P, C], FP32, tag="ps")
            for mc in range(CC):
                nc.tensor.transpose(
                    pt[:, mc * P:(mc + 1) * P],
                    _f32(attn_sb[:, mc, dc * P:(dc + 1) * P]),
                    identity,
                )
            nc.any.tensor_copy(attnT_sb[:, dc, :], pt)

        # ---- A'^T[j, c] = sum_d Wg[d, j] attn[c, d]
        apT_sb = scratch.tile([P, CC, C], FP32R, tag="s8")
        for jc in range(CC):
            ap_ps = ps.tile([P, C], FP32, tag="ps")
            for dc in range(CC):
                nc.tensor.matmul(
                    ap_ps,
                    wg_sb[:, dc, jc * P:(jc + 1) * P],
                    attnT_sb[:, dc, :],
                    start=(dc == 0),
                    stop=(dc == CC - 1),
                )
            nc.any.tensor_copy(apT_sb[:, jc, :], ap_ps)

        # ---- y^T blocks straight into Z layout, interleaved q-major with the
        # final mask GEMM + residual + store.
        # Z[i, q*512 + r] = y^T[8*i + q, r]; with n = 1024*ci + 8*m + q the
        # output PSUM tile [m, r] equals ZS[:, ci, q*512:(q+1)*512], and the
        # mask GEMM for output block jb=q only needs ZS blocks (ci=0..3, q).
        zs_sb = bigpool.tile([P, CC, N], FP32R, tag="big")
        for q in range(QF):
            for ci in range(CC):
                zp = ps.tile([P, C], FP32, tag="ps")
                for jc in range(CC):
                    xr = x_sb[:, jc, :].rearrange(
                        "p (ci m q) -> p ci q m", ci=CC, q=QF
                    )
                    nc.tensor.matmul(
                        zp,
                        xr[:, ci, q, :],
                        apT_sb[:, jc, :],
                        start=(jc == 0),
                        stop=(jc == CC - 1),
                    )
                nc.any.tensor_copy(zs_sb[:, ci, q * C:(q + 1) * C], zp)

            jb = q
            for oc in range(CC):
                mp = psg.tile([P, C], FP32, tag="gacc")
                for ic in range(CC):
                    nc.tensor.matmul(
                        mp,
                        wmT[:, ic, oc * P:(oc + 1) * P],
                        zs_sb[:, ic, jb * C:(jb + 1) * C],
                        start=(ic == 0),
                        stop=(ic == CC - 1),
                    )
                ot = outp.tile([P, C], FP32)
                nc.vector.tensor_add(
                    ot, mp, _f32(x_sb[:, oc, jb * C:(jb + 1) * C])
                )
                nc.sync.dma_start(
                    out=out_d[oc * P:(oc + 1) * P, jb * C:(jb + 1) * C], in_=ot
                )


_NC_CACHE = {}
LAST_RESULT = None


def get_nc():
    if "nc" not in _NC_CACHE:
        _NC_CACHE["nc"] = _build_nc()
    return _NC_CACHE["nc"]


def _round_fp32r(x):
    """Round fp32 array to the fp32r grid (11 explicit mantissa bits, RNE)."""
    u = np.ascontiguousarray(x, dtype=np.float32).view(np.uint32).astype(np.uint64)
    shift = 23 - 11
    add = (np.uint64(1) << np.uint64(shift - 1)) - np.uint64(1) + (
        (u >> np.uint64(shift)) & np.uint64(1)
    )
    u = (u + add) & np.uint64(~((1 << shift) - 1) & 0xFFFFFFFF)
    return u.astype(np.uint32).view(np.float32)


def make_in_map(xb, w_phi_t, w_theta_t, w_g, w_mask_t_g):
    """Per-core input dict; xb is one sample [C, H, W]."""
    xr = _round_fp32r(xb.reshape(C, N))
    return {
        "x": xr,
        "xt": np.ascontiguousarray(xr.T),
        "w_phi_t": w_phi_t,
        "w_theta_t": w_theta_t,
        "w_g": w_g,
        "w_mask_t_g": w_mask_t_g,
    }


def prep_weights(w_phi, w_theta, w_g, w_mask, gamma):
    w_phi_t = _round_fp32r(np.asarray(w_phi, dtype=np.float32).T)
    w_theta_t = _round_fp32r(np.asarray(w_theta, dtype=np.float32).T)
    w_g_r = _round_fp32r(np.asarray(w_g, dtype=np.float32))
    gamma64 = float(np.asarray(gamma, dtype=np.float32).reshape(-1)[0])
    w_mask_t_g = _round_fp32r(
        (np.asarray(w_mask, dtype=np.float64).T * gamma64).astype(np.float32)
    )
    return w_phi_t, w_theta_t, w_g_r, w_mask_t_g


def kernel(x, w_phi, w_theta, w_g, w_mask, gamma):
    global LAST_RESULT
    x = np.ascontiguousarray(np.asarray(x, dtype=np.float32))
    B, c, h, w = x.shape
    assert (c, h * w) == (C, N), (x.shape,)

    w_phi_t, w_theta_t, w_g_r, w_mask_t_g = prep_weights(
        w_phi, w_theta, w_g, w_mask, gamma
    )
    nc = get_nc()
    in_maps = [
        make_in_map(x[b], w_phi_t, w_theta_t, w_g_r, w_mask_t_g)
        for b in range(B)
    ]
    trace = bool(int(os.environ.get("KERNEL_TRACE", "0")))
    res = run_bass_kernel_spmd(nc, in_maps, list(range(B)), trace=trace)
    LAST_RESULT = res
    out = np.stack([res.results[b]["out"].reshape(c, h, w) for b in range(B)])
    return out


# revision 10
# speedup vs baseline: 1.1570x; 1.0312x over previous
# Trainium2 Bass kernel for per-sample channel-attention module (CAM).
#
# Reference math per sample (C=512, N=H*W=4096):
#   X = x.reshape(C, N)
#   phi = Wp X ; theta = Wt X ; g = Wg X
#   attn = softmax_rows(phi @ theta^T)          # [C, C]
#   y = attn @ g                                 # [C, N]
#   Z = (y^T).flatten().reshape(C, N)            # torch permute+view reinterpretation
#   out = gamma * (Wm @ Z) + x
#
# Algebraic restructuring (cuts PE work ~1.8x vs the naive 6-GEMM chain):
#   G = X X^T                  (Gram, [C, C])
#   L = Wp G Wt^T              (attention logits via two small GEMMs)
#   A' = softmax(L) @ Wg       (fold g-projection into attention)
#   y = A' X                   (single big GEMM)
# The torch permute+view reinterpretation is free: y^T blocks are produced
# with a stride-8 column selection of X as the stationary matmul operand, so
# each PSUM tile lands exactly on a contiguous block of Z's SBUF layout.
#
# All matmuls run in float32r (fp32 data streamed through the PE at
# 1 row/cycle; operands carry 11 explicit mantissa bits, RNE). The host
# pre-rounds inputs to the fp32r grid and also pre-computes pure layout
# transforms: X^T, Wp^T, Wt^T, and gamma*Wm^T (zero FLOPs of the reference
# are moved off-device; every GEMM/softmax runs on the NeuronCore).

import os
import numpy as np

import concourse.bass as bass
import concourse.mybir as mybir
import concourse.tile as tile
from concourse import bacc
from concourse.bass_utils import run_bass_kernel_spmd
from concourse.tile import TileContext
from concourse.masks import make_identity

P = 128          # partitions
C = 512          # channels
N = 4096         # spatial (64*64)
CC = C // P      # 4 channel chunks
NT = N // P      # 32 spatial tiles
QF = N // C      # 8 fold factor for the permute+view reinterpretation
FP32 = mybir.dt.float32
FP32R = mybir.dt.float32r


def _f32(ap):
    # reinterpret an fp32r tile as plain fp32 (identical bit layout)
    return ap.bitcast(FP32)


def _build_nc():
    nc = bacc.Bacc("TRN2", target_bir_lowering=False, debug=False, num_devices=8)
    x_d = nc.dram_tensor("x", [C, N], FP32R, kind="ExternalInput").ap()
    xt_d = nc.dram_tensor("xt", [N, C], FP32R, kind="ExternalInput").ap()
    wphiT_d = nc.dram_tensor("w_phi_t", [C, C], FP32R, kind="ExternalInput").ap()
    wthetaT_d = nc.dram_tensor("w_theta_t", [C, C], FP32R, kind="ExternalInput").ap()
    wg_d = nc.dram_tensor("w_g", [C, C], FP32R, kind="ExternalInput").ap()
    wmTg_d = nc.dram_tensor("w_mask_t_g", [C, C], FP32R, kind="ExternalInput").ap()
    out_d = nc.dram_tensor("out", [C, N], FP32, kind="ExternalOutput").ap()

    with TileContext(nc) as tc:
        _body(tc, x_d, xt_d, wphiT_d, wthetaT_d, wg_d, wmTg_d, out_d)
    nc.compile()
    return nc


def _body(tc, x_d, xt_d, wphiT_d, wthetaT_d, wg_d, wmTg_d, out_d):
    nc = tc.nc
    from contextlib import ExitStack

    with ExitStack() as ctx:
        const = ctx.enter_context(tc.tile_pool(name="const", bufs=1))
        xpool = ctx.enter_context(tc.tile_pool(name="xpool", bufs=1))
        wpool = ctx.enter_context(tc.tile_pool(name="wpool", bufs=1))
        bigpool = ctx.enter_context(tc.tile_pool(name="bigpool", bufs=1))
        scratch = ctx.enter_context(tc.tile_pool(name="scratch", bufs=2))
        vecs = ctx.enter_context(tc.tile_pool(name="vecs", bufs=8))
        outp = ctx.enter_context(tc.tile_pool(name="outp", bufs=4))
        ps = ctx.enter_context(tc.tile_pool(name="ps", bufs=4, space="PSUM"))
        psg = ctx.enter_context(tc.tile_pool(name="psg", bufs=4, space="PSUM"))

        identity = const.tile([P, P], FP32)
        make_identity(nc, identity)

        # ---- weight loads on the SWDGE path (gpsimd) so they never delay the
        # latency-critical xt stream on the Sync HWDGE queue.
        # Layout [p, cc, j]: tile[p, cc, j] = W[128*cc + p, j].
        wphiT = wpool.tile([P, CC, C], FP32R)
        wthetaT = wpool.tile([P, CC, C], FP32R)
        wg_sb = wpool.tile([P, CC, C], FP32R)
        wmT = wpool.tile([P, CC, C], FP32R)
        # Weight + x loads go on the ACT HWDGE queue (nc.scalar.dma_start)
        # so the latency-critical xt stream owns the Sync queue. Order by
        # first use: wthetaT (T1), wphiT (L), wg (A'), then x, then wmT.
        # x arrives in column-quarters: ZS pass ci only reads columns
        # [1024*ci, 1024*(ci+1)), so quarter ci unblocks that pass.
        x_sb = xpool.tile([P, CC, N], FP32R)
        QW = N // CC  # 1024

        def _load_x_quarter(ci):
            nc.scalar.dma_start(
                out=x_sb[:, :, ci * QW:(ci + 1) * QW],
                in_=x_d[:, ci * QW:(ci + 1) * QW].rearrange(
                    "(cc p) n -> p cc n", p=P
                ),
            )

        nc.scalar.dma_start(
            out=wthetaT, in_=wthetaT_d.rearrange("(cc p) j -> p cc j", p=P)
        )
        nc.scalar.dma_start(
            out=wphiT, in_=wphiT_d.rearrange("(cc p) j -> p cc j", p=P)
        )
        _load_x_quarter(0)
        nc.scalar.dma_start(
            out=wg_sb, in_=wg_d.rearrange("(cc p) j -> p cc j", p=P)
        )
        _load_x_quarter(1)
        nc.scalar.dma_start(
            out=wmT, in_=wmTg_d.rearrange("(cc p) j -> p cc j", p=P)
        )
        _load_x_quarter(2)
        _load_x_quarter(3)

        # ---- stream X^T in 1 MB chunks (4 tiles each) and fold each tile
        # into the Gram accumulators as soon as its chunk lands.
        # XT[p, t, c] = X[c, 128*t + p];  G[a, b] = sum_n X[a, n] X[b, n].
        xt_sb = bigpool.tile([P, NT, C], FP32R, tag="big")
        gacc = [
            psg.tile([P, C], FP32, tag="gacc", name=f"gacc{i}")
            for i in range(CC)
        ]
        for tg in range(NT // 4):
            nc.sync.dma_start(
                out=xt_sb[:, tg * 4:(tg + 1) * 4, :],
                in_=xt_d[tg * 4 * P:(tg + 1) * 4 * P, :].rearrange(
                    "(tt p) c -> p tt c", p=P
                ),
            )
            for k in range(4):
                t = tg * 4 + k
                for mc in range(CC):
                    nc.tensor.matmul(
                        gacc[mc],
                        xt_sb[:, t, mc * P:(mc + 1) * P],
                        xt_sb[:, t, :],
                        start=(t == 0),
                        stop=(t == NT - 1),
                    )

        g_sb = scratch.tile([P, CC, C], FP32R, tag="s8")
        for mc in range(CC):
            nc.any.tensor_copy(g_sb[:, mc, :], gacc[mc])

        # ---- T1 = G @ Wt^T  (uses G symmetry for the stationary operand)
        t1_sb = scratch.tile([P, CC, C], FP32R, tag="s8")
        for mc in range(CC):
            tp = ps.tile([P, C], FP32, tag="ps")
            for jc in range(CC):
                nc.tensor.matmul(
                    tp,
                    g_sb[:, jc, mc * P:(mc + 1) * P],
                    wthetaT[:, jc, :],
                    start=(jc == 0),
                    stop=(jc == CC - 1),
                )
            nc.any.tensor_copy(t1_sb[:, mc, :], tp)

        # ---- L = Wp @ T1 ; softmax rows -> attn
        attn_sb = scratch.tile([P, CC, C], FP32R, tag="s8")
        for mc in range(CC):
            lp = ps.tile([P, C], FP32, tag="ps")
            for ic in range(CC):
                nc.tensor.matmul(
                    lp,
                    wphiT[:, ic, mc * P:(mc + 1) * P],
                    t1_sb[:, ic, :],
                    start=(ic == 0),
                    stop=(ic == CC - 1),
                )
            neg_max = vecs.tile([P, 1], FP32)
            nc.vector.tensor_reduce(
                out=neg_max, in_=lp, axis=mybir.AxisListType.X,
                op=mybir.AluOpType.max, negate=True,
            )
            sums = vecs.tile([P, 1], FP32)
            nc.scalar.activation(
                out=attn_sb[:, mc, :], in_=lp,
                func=mybir.ActivationFunctionType.Exp,
                bias=neg_max, scale=1.0, accum_out=sums,
            )
            rinv = vecs.tile([P, 1], FP32)
            nc.vector.reciprocal(rinv, sums)
            nc.vector.tensor_scalar_mul(
                attn_sb[:, mc, :], attn_sb[:, mc, :], rinv
            )

        # ---- attn^T via PE transposes (fp32 mode; copies round to fp32r)
        attnT_sb = scratch.tile([P, CC, C], FP32R, tag="s8")
        for dc in range(CC):
            pt = ps.tile([P, C], FP32, tag="ps")
            for mc in range(CC):
                nc.tensor.transpose(
                    pt[:, mc * P:(mc + 1) * P],
                    _f32(attn_sb[:, mc, dc * P:(dc + 1) * P]),
                    identity,
                )
            nc.any.tensor_copy(attnT_sb[:, dc, :], pt)

        # ---- A'^T[j, c] = sum_d Wg[d, j] attn[c, d]
        apT_sb = scratch.tile([P, CC, C], FP32R, tag="s8")
        for jc in range(CC):
            ap_ps = ps.tile([P, C], FP32, tag="ps")
            for dc in range(CC):
                nc.tensor.matmul(
                    ap_ps,
                    wg_sb[:, dc, jc * P:(jc + 1) * P],
                    attnT_sb[:, dc, :],
                    start=(dc == 0),
                    stop=(dc == CC - 1),
                )
            nc.any.tensor_copy(apT_sb[:, jc, :], ap_ps)

        # ---- y^T blocks straight into Z layout, interleaved q-major with the
        # final mask GEMM + residual + store.
        # Z[i, q*512 + r] = y^T[8*i + q, r]; with n = 1024*ci + 8*m + q the
        # output PSUM tile [m, r] equals ZS[:, ci, q*512:(q+1)*512], and the
        # mask GEMM for output block jb=q only needs ZS blocks (ci=0..3, q).
        zs_sb = bigpool.tile([P, CC, N], FP32R, tag="big")
        for ci in range(CC):
            for q in range(QF):
                zp = ps.tile([P, C], FP32, tag="ps")
                for jc in range(CC):
                    xr = x_sb[:, jc, :].rearrange(
                        "p (ci m q) -> p ci q m", ci=CC, q=QF
                    )
                    nc.tensor.matmul(
                        zp,
                        xr[:, ci, q, :],
                        apT_sb[:, jc, :],
                        start=(jc == 0),
                        stop=(jc == CC - 1),
                    )
                nc.any.tensor_copy(zs_sb[:, ci, q * C:(q + 1) * C], zp)

                if ci == CC - 1:
                    # ZS blocks (0..3, q) are now all done: emit output block q
                    jb = q
                    for oc in range(CC):
                        mp = psg.tile([P, C], FP32, tag="gacc")
                        for ic in range(CC):
                            nc.tensor.matmul(
                                mp,
                                wmT[:, ic, oc * P:(oc + 1) * P],
                                zs_sb[:, ic, jb * C:(jb + 1) * C],
                                start=(ic == 0),
                                stop=(ic == CC - 1),
                            )
                        ot = outp.tile([P, C], FP32)
                        nc.vector.tensor_add(
                            ot, mp, _f32(x_sb[:, oc, jb * C:(jb + 1) * C])
                        )
                        nc.sync.dma_start(
                            out=out_d[oc * P:(oc + 1) * P, jb * C:(jb + 1) * C],
                            in_=ot,
                        )


_NC_CACHE = {}
LAST_RESULT = None


def get_nc():
    if "nc" not in _NC_CACHE:
        _NC_CACHE["nc"] = _build_nc()
    return _NC_CACHE["nc"]


def _round_fp32r(x):
    """Round fp32 array to the fp32r grid (11 explicit mantissa bits, RNE)."""
    u = np.ascontiguousarray(x, dtype=np.float32).view(np.uint32).astype(np.uint64)
    shift = 23 - 11
    add = (np.uint64(1) << np.uint64(shift - 1)) - np.uint64(1) + (
        (u >> np.uint64(shift)) & np.uint64(1)
    )
    u = (u + add) & np.uint64(~((1 << shift) - 1) & 0xFFFFFFFF)
    return u.astype(np.uint32).view(np.float32)


def make_in_map(xb, w_phi_t, w_theta_t, w_g, w_mask_t_g):
    """Per-core input dict; xb is one sample [C, H, W]."""
    xr = _round_fp32r(xb.reshape(C, N))
    return {
        "x": xr,
        "xt": np.ascontiguousarray(xr.T),
        "w_phi_t": w_phi_t,
        "w_theta_t": w_theta_t,
        "w_g": w_g,
        "w_mask_t_g": w_mask_t_g,
    }


def prep_weights(w_phi, w_theta, w_g, w_mask, gamma):
    w_phi_t = _round_fp32r(np.asarray(w_phi, dtype=np.float32).T)
    w_theta_t = _round_fp32r(np.asarray(w_theta, dtype=np.float32).T)
    w_g_r = _round_fp32r(np.asarray(w_g, dtype=np.float32))
    gamma64 = float(np.asarray(gamma, dtype=np.float32).reshape(-1)[0])
    w_mask_t_g = _round_fp32r(
        (np.asarray(w_mask, dtype=np.float64).T * gamma64).astype(np.float32)
    )
    return w_phi_t, w_theta_t, w_g_r, w_mask_t_g


def kernel(x, w_phi, w_theta, w_g, w_mask, gamma):
    global LAST_RESULT
    x = np.ascontiguousarray(np.asarray(x, dtype=np.float32))
    B, c, h, w = x.shape
    assert (c, h * w) == (C, N), (x.shape,)

    w_phi_t, w_theta_t, w_g_r, w_mask_t_g = prep_weights(
        w_phi, w_theta, w_g, w_mask, gamma
    )
    nc = get_nc()
    in_maps = [
        make_in_map(x[b], w_phi_t, w_theta_t, w_g_r, w_mask_t_g)
        for b in range(B)
    ]
    trace = bool(int(os.environ.get("KERNEL_TRACE", "0")))
    res = run_bass_kernel_spmd(nc, in_maps, list(range(B)), trace=trace)
    LAST_RESULT = res
    out = np.stack([res.results[b]["out"].reshape(c, h, w) for b in range(B)])
    return out


# revision 12
# speedup vs baseline: 1.1837x; 1.0231x over previous
# Trainium2 Bass kernel for per-sample channel-attention module (CAM).
#
# Reference math per sample (C=512, N=H*W=4096):
#   X = x.reshape(C, N)
#   phi = Wp X ; theta = Wt X ; g = Wg X
#   attn = softmax_rows(phi @ theta^T)          # [C, C]
#   y = attn @ g                                 # [C, N]
#   Z = (y^T).flatten().reshape(C, N)            # torch permute+view reinterpretation
#   out = gamma * (Wm @ Z) + x
#
# Algebraic restructuring (cuts PE work ~1.8x vs the naive 6-GEMM chain):
#   G = X X^T                  (Gram, [C, C])
#   L = Wp G Wt^T              (attention logits via two small GEMMs)
#   A' = softmax(L) @ Wg       (fold g-projection into attention)
#   y = A' X                   (single big GEMM)
# The torch permute+view reinterpretation is free: y^T blocks are produced
# with a stride-8 column selection of X as the stationary matmul operand, so
# each PSUM tile lands exactly on a contiguous block of Z's SBUF layout.
#
# All matmuls run in float32r (fp32 data streamed through the PE at
# 1 row/cycle; operands carry 11 explicit mantissa bits, RNE). The host
# pre-rounds inputs to the fp32r grid and also pre-computes pure layout
# transforms: X^T, Wp^T, Wt^T, and gamma*Wm^T (zero FLOPs of the reference
# are moved off-device; every GEMM/softmax runs on the NeuronCore).

import os
import numpy as np

import concourse.bass as bass
import concourse.mybir as mybir
import concourse.tile as tile
from concourse import bacc
from concourse.bass_utils import run_bass_kernel_spmd
from concourse.tile import TileContext
from concourse.masks import make_identity

P = 128          # partitions
C = 512          # channels
N = 4096         # spatial (64*64)
CC = C // P      # 4 channel chunks
NT = N // P      # 32 spatial tiles
QF = N // C      # 8 fold factor for the permute+view reinterpretation
FP32 = mybir.dt.float32
FP32R = mybir.dt.float32r


def _f32(ap):
    # reinterpret an fp32r tile as plain fp32 (identical bit layout)
    return ap.bitcast(FP32)


def _build_nc():
    nc = bacc.Bacc("TRN2", target_bir_lowering=False, debug=False, num_devices=8)
    x_d = nc.dram_tensor("x", [C, N], FP32R, kind="ExternalInput").ap()
    xt_d = nc.dram_tensor("xt", [N, C], FP32R, kind="ExternalInput").ap()
    wphiT_d = nc.dram_tensor("w_phi_t", [C, C], FP32R, kind="ExternalInput").ap()
    wthetaT_d = nc.dram_tensor("w_theta_t", [C, C], FP32R, kind="ExternalInput").ap()
    wg_d = nc.dram_tensor("w_g", [C, C], FP32R, kind="ExternalInput").ap()
    wmTg_d = nc.dram_tensor("w_mask_t_g", [C, C], FP32R, kind="ExternalInput").ap()
    out_d = nc.dram_tensor("out", [C, N], FP32, kind="ExternalOutput").ap()

    with TileContext(nc) as tc:
        _body(tc, x_d, xt_d, wphiT_d, wthetaT_d, wg_d, wmTg_d, out_d)
    nc.compile()
    return nc


def _body(tc, x_d, xt_d, wphiT_d, wthetaT_d, wg_d, wmTg_d, out_d):
    nc = tc.nc
    from contextlib import ExitStack

    with ExitStack() as ctx:
        const = ctx.enter_context(tc.tile_pool(name="const", bufs=1))
        xpool = ctx.enter_context(tc.tile_pool(name="xpool", bufs=1))
        wpool = ctx.enter_context(tc.tile_pool(name="wpool", bufs=1))
        bigpool = ctx.enter_context(tc.tile_pool(name="bigpool", bufs=1))
        scratch = ctx.enter_context(tc.tile_pool(name="scratch", bufs=2))
        vecs = ctx.enter_context(tc.tile_pool(name="vecs", bufs=8))
        outp = ctx.enter_context(tc.tile_pool(name="outp", bufs=4))
        ps = ctx.enter_context(tc.tile_pool(name="ps", bufs=4, space="PSUM"))
        psg = ctx.enter_context(tc.tile_pool(name="psg", bufs=4, space="PSUM"))

        identity = const.tile([P, P], FP32)
        make_identity(nc, identity)

        # ~12 throwaway matmuls warm the PE (HAM un-throttles after ~3.4 us
        # of activity) while the first xt chunk is still in flight.
        warm = psg.tile([P, P], FP32, tag="gacc")
        for _ in range(12):
            nc.tensor.matmul(warm, identity, identity, start=True, stop=True)

        # ---- weight loads on the SWDGE path (gpsimd) so they never delay the
        # latency-critical xt stream on the Sync HWDGE queue.
        # Layout [p, cc, j]: tile[p, cc, j] = W[128*cc + p, j].
        wphiT = wpool.tile([P, CC, C], FP32R)
        wthetaT = wpool.tile([P, CC, C], FP32R)
        wg_sb = wpool.tile([P, CC, C], FP32R)
        wmT = wpool.tile([P, CC, C], FP32R)
        # Weight + x loads go on the ACT HWDGE queue (nc.scalar.dma_start)
        # so the latency-critical xt stream owns the Sync queue. Order by
        # first use: wthetaT (T1), wphiT (L), wg (A'), then x, then wmT.
        # x arrives in column-quarters: ZS pass ci only reads columns
        # [1024*ci, 1024*(ci+1)), so quarter ci unblocks that pass.
        x_sb = xpool.tile([P, CC, N], FP32R)
        QW = N // CC  # 1024

        def _load_x_quarter(ci):
            nc.scalar.dma_start(
                out=x_sb[:, :, ci * QW:(ci + 1) * QW],
                in_=x_d[:, ci * QW:(ci + 1) * QW].rearrange(
                    "(cc p) n -> p cc n", p=P
                ),
            )

        nc.scalar.dma_start(
            out=wthetaT, in_=wthetaT_d.rearrange("(cc p) j -> p cc j", p=P)
        )
        nc.scalar.dma_start(
            out=wphiT, in_=wphiT_d.rearrange("(cc p) j -> p cc j", p=P)
        )
        _load_x_quarter(0)
        nc.scalar.dma_start(
            out=wg_sb, in_=wg_d.rearrange("(cc p) j -> p cc j", p=P)
        )
        _load_x_quarter(1)
        _load_x_quarter(2)
        _load_x_quarter(3)
        nc.scalar.dma_start(
            out=wmT, in_=wmTg_d.rearrange("(cc p) j -> p cc j", p=P)
        )

        # ---- stream X^T in 1 MB chunks (4 tiles each) and fold each tile
        # into the Gram accumulators as soon as its chunk lands.
        # XT[p, t, c] = X[c, 128*t + p];  G[a, b] = sum_n X[a, n] X[b, n].
        xt_sb = bigpool.tile([P, NT, C], FP32R, tag="big")
        gacc = [
            psg.tile([P, C], FP32, tag="gacc", name=f"gacc{i}")
            for i in range(CC)
        ]
        for tg in range(NT // 4):
            nc.sync.dma_start(
                out=xt_sb[:, tg * 4:(tg + 1) * 4, :],
                in_=xt_d[tg * 4 * P:(tg + 1) * 4 * P, :].rearrange(
                    "(tt p) c -> p tt c", p=P
                ),
            )
            for k in range(4):
                t = tg * 4 + k
                for mc in range(CC):
                    nc.tensor.matmul(
                        gacc[mc],
                        xt_sb[:, t, mc * P:(mc + 1) * P],
                        xt_sb[:, t, :],
                        start=(t == 0),
                        stop=(t == NT - 1),
                    )

        g_sb = scratch.tile([P, CC, C], FP32R, tag="s8")
        for mc in range(CC):
            nc.any.tensor_copy(g_sb[:, mc, :], gacc[mc])

        # ---- T1 = G @ Wt^T  (uses G symmetry for the stationary operand)
        t1_sb = scratch.tile([P, CC, C], FP32R, tag="s8")
        for mc in range(CC):
            tp = ps.tile([P, C], FP32, tag="ps")
            for jc in range(CC):
                nc.tensor.matmul(
                    tp,
                    g_sb[:, jc, mc * P:(mc + 1) * P],
                    wthetaT[:, jc, :],
                    start=(jc == 0),
                    stop=(jc == CC - 1),
                )
            nc.any.tensor_copy(t1_sb[:, mc, :], tp)

        # ---- L = Wp @ T1 ; softmax rows -> attn
        attn_sb = scratch.tile([P, CC, C], FP32R, tag="s8")
        for mc in range(CC):
            lp = ps.tile([P, C], FP32, tag="ps")
            for ic in range(CC):
                nc.tensor.matmul(
                    lp,
                    wphiT[:, ic, mc * P:(mc + 1) * P],
                    t1_sb[:, ic, :],
                    start=(ic == 0),
                    stop=(ic == CC - 1),
                )
            neg_max = vecs.tile([P, 1], FP32)
            nc.vector.tensor_reduce(
                out=neg_max, in_=lp, axis=mybir.AxisListType.X,
                op=mybir.AluOpType.max, negate=True,
            )
            sums = vecs.tile([P, 1], FP32)
            nc.scalar.activation(
                out=attn_sb[:, mc, :], in_=lp,
                func=mybir.ActivationFunctionType.Exp,
                bias=neg_max, scale=1.0, accum_out=sums,
            )
            rinv = vecs.tile([P, 1], FP32)
            nc.vector.reciprocal(rinv, sums)
            nc.vector.tensor_scalar_mul(
                attn_sb[:, mc, :], attn_sb[:, mc, :], rinv
            )

        # ---- attn^T via PE transposes (fp32 mode; copies round to fp32r)
        attnT_sb = scratch.tile([P, CC, C], FP32R, tag="s8")
        for dc in range(CC):
            pt = ps.tile([P, C], FP32, tag="ps")
            for mc in range(CC):
                nc.tensor.transpose(
                    pt[:, mc * P:(mc + 1) * P],
                    _f32(attn_sb[:, mc, dc * P:(dc + 1) * P]),
                    identity,
                )
            nc.any.tensor_copy(attnT_sb[:, dc, :], pt)

        # ---- A'^T[j, c] = sum_d Wg[d, j] attn[c, d]
        apT_sb = scratch.tile([P, CC, C], FP32R, tag="s8")
        for jc in range(CC):
            ap_ps = ps.tile([P, C], FP32, tag="ps")
            for dc in range(CC):
                nc.tensor.matmul(
                    ap_ps,
                    wg_sb[:, dc, jc * P:(jc + 1) * P],
                    attnT_sb[:, dc, :],
                    start=(dc == 0),
                    stop=(dc == CC - 1),
                )
            nc.any.tensor_copy(apT_sb[:, jc, :], ap_ps)

        # ---- y^T blocks straight into Z layout, interleaved q-major with the
        # final mask GEMM + residual + store.
        # Z[i, q*512 + r] = y^T[8*i + q, r]; with n = 1024*ci + 8*m + q the
        # output PSUM tile [m, r] equals ZS[:, ci, q*512:(q+1)*512], and the
        # mask GEMM for output block jb=q only needs ZS blocks (ci=0..3, q).
        zs_sb = bigpool.tile([P, CC, N], FP32R, tag="big")
        for ci in range(CC):
            for q in range(QF):
                zp = ps.tile([P, C], FP32, tag="ps")
                for jc in range(CC):
                    xr = x_sb[:, jc, :].rearrange(
                        "p (ci m q) -> p ci q m", ci=CC, q=QF
                    )
                    nc.tensor.matmul(
                        zp,
                        xr[:, ci, q, :],
                        apT_sb[:, jc, :],
                        start=(jc == 0),
                        stop=(jc == CC - 1),
                    )
                nc.any.tensor_copy(zs_sb[:, ci, q * C:(q + 1) * C], zp)

                if ci == CC - 1:
                    # ZS blocks (0..3, q) are now all done: emit output block q
                    jb = q
                    for oc in range(CC):
                        mp = psg.tile([P, C], FP32, tag="gacc")
                        for ic in range(CC):
                            nc.tensor.matmul(
                                mp,
                                wmT[:, ic, oc * P:(oc + 1) * P],
                                zs_sb[:, ic, jb * C:(jb + 1) * C],
                                start=(ic == 0),
                                stop=(ic == CC - 1),
                            )
                        ot = outp.tile([P, C], FP32)
                        nc.vector.tensor_add(
                            ot, mp, _f32(x_sb[:, oc, jb * C:(jb + 1) * C])
                        )
                        nc.sync.dma_start(
                            out=out_d[oc * P:(oc + 1) * P, jb * C:(jb + 1) * C],
                            in_=ot,
                        )


_NC_CACHE = {}
LAST_RESULT = None


def get_nc():
    if "nc" not in _NC_CACHE:
        _NC_CACHE["nc"] = _build_nc()
    return _NC_CACHE["nc"]


def _round_fp32r(x):
    """Round fp32 array to the fp32r grid (11 explicit mantissa bits, RNE)."""
    u = np.ascontiguousarray(x, dtype=np.float32).view(np.uint32).astype(np.uint64)
    shift = 23 - 11
    add = (np.uint64(1) << np.uint64(shift - 1)) - np.uint64(1) + (
        (u >> np.uint64(shift)) & np.uint64(1)
    )
    u = (u + add) & np.uint64(~((1 << shift) - 1) & 0xFFFFFFFF)
    return u.astype(np.uint32).view(np.float32)


def make_in_map(xb, w_phi_t, w_theta_t, w_g, w_mask_t_g):
    """Per-core input dict; xb is one sample [C, H, W]."""
    xr = _round_fp32r(xb.reshape(C, N))
    return {
        "x": xr,
        "xt": np.ascontiguousarray(xr.T),
        "w_phi_t": w_phi_t,
        "w_theta_t": w_theta_t,
        "w_g": w_g,
        "w_mask_t_g": w_mask_t_g,
    }


def prep_weights(w_phi, w_theta, w_g, w_mask, gamma):
    w_phi_t = _round_fp32r(np.asarray(w_phi, dtype=np.float32).T)
    w_theta_t = _round_fp32r(np.asarray(w_theta, dtype=np.float32).T)
    w_g_r = _round_fp32r(np.asarray(w_g, dtype=np.float32))
    gamma64 = float(np.asarray(gamma, dtype=np.float32).reshape(-1)[0])
    w_mask_t_g = _round_fp32r(
        (np.asarray(w_mask, dtype=np.float64).T * gamma64).astype(np.float32)
    )
    return w_phi_t, w_theta_t, w_g_r, w_mask_t_g


def kernel(x, w_phi, w_theta, w_g, w_mask, gamma):
    global LAST_RESULT
    x = np.ascontiguousarray(np.asarray(x, dtype=np.float32))
    B, c, h, w = x.shape
    assert (c, h * w) == (C, N), (x.shape,)

    w_phi_t, w_theta_t, w_g_r, w_mask_t_g = prep_weights(
        w_phi, w_theta, w_g, w_mask, gamma
    )
    nc = get_nc()
    in_maps = [
        make_in_map(x[b], w_phi_t, w_theta_t, w_g_r, w_mask_t_g)
        for b in range(B)
    ]
    trace = bool(int(os.environ.get("KERNEL_TRACE", "0")))
    res = run_bass_kernel_spmd(nc, in_maps, list(range(B)), trace=trace)
    LAST_RESULT = res
    out = np.stack([res.results[b]["out"].reshape(c, h, w) for b in range(B)])
    return out


# revision 13
# speedup vs baseline: 1.1885x; 1.0040x over previous
# Trainium2 Bass kernel for per-sample channel-attention module (CAM).
#
# Reference math per sample (C=512, N=H*W=4096):
#   X = x.reshape(C, N)
#   phi = Wp X ; theta = Wt X ; g = Wg X
#   attn = softmax_rows(phi @ theta^T)          # [C, C]
#   y = attn @ g                                 # [C, N]
#   Z = (y^T).flatten().reshape(C, N)            # torch permute+view reinterpretation
#   out = gamma * (Wm @ Z) + x
#
# Algebraic restructuring (cuts PE work ~1.8x vs the naive 6-GEMM chain):
#   G = X X^T                  (Gram, [C, C])
#   L = Wp G Wt^T              (attention logits via two small GEMMs)
#   A' = softmax(L) @ Wg       (fold g-projection into attention)
#   y = A' X                   (single big GEMM)
# The torch permute+view reinterpretation is free: y^T blocks are produced
# with a stride-8 column selection of X as the stationary matmul operand, so
# each PSUM tile lands exactly on a contiguous block of Z's SBUF layout.
#
# All matmuls run in float32r (fp32 data streamed through the PE at
# 1 row/cycle; operands carry 11 explicit mantissa bits, RNE). The host
# pre-rounds inputs to the fp32r grid and also pre-computes pure layout
# transforms: X^T, Wp^T, Wt^T, and gamma*Wm^T (zero FLOPs of the reference
# are moved off-device; every GEMM/softmax runs on the NeuronCore).

import os
import numpy as np

import concourse.bass as bass
import concourse.mybir as mybir
import concourse.tile as tile
from concourse import bacc
from concourse.bass_utils import run_bass_kernel_spmd
from concourse.tile import TileContext
from concourse.masks import make_identity

P = 128          # partitions
C = 512          # channels
N = 4096         # spatial (64*64)
CC = C // P      # 4 channel chunks
NT = N // P      # 32 spatial tiles
QF = N // C      # 8 fold factor for the permute+view reinterpretation
FP32 = mybir.dt.float32
FP32R = mybir.dt.float32r


def _f32(ap):
    # reinterpret an fp32r tile as plain fp32 (identical bit layout)
    return ap.bitcast(FP32)


def _build_nc():
    nc = bacc.Bacc("TRN2", target_bir_lowering=False, debug=False, num_devices=8)
    x_d = nc.dram_tensor("x", [C, N], FP32R, kind="ExternalInput").ap()
    xt_d = nc.dram_tensor("xt", [N, C], FP32R, kind="ExternalInput").ap()
    wphiT_d = nc.dram_tensor("w_phi_t", [C, C], FP32R, kind="ExternalInput").ap()
    wthetaT_d = nc.dram_tensor("w_theta_t", [C, C], FP32R, kind="ExternalInput").ap()
    wg_d = nc.dram_tensor("w_g", [C, C], FP32R, kind="ExternalInput").ap()
    wmTg_d = nc.dram_tensor("w_mask_t_g", [C, C], FP32R, kind="ExternalInput").ap()
    out_d = nc.dram_tensor("out", [C, N], FP32, kind="ExternalOutput").ap()

    with TileContext(nc) as tc:
        _body(tc, x_d, xt_d, wphiT_d, wthetaT_d, wg_d, wmTg_d, out_d)
    nc.compile()
    return nc


def _body(tc, x_d, xt_d, wphiT_d, wthetaT_d, wg_d, wmTg_d, out_d):
    nc = tc.nc
    from contextlib import ExitStack

    with ExitStack() as ctx:
        const = ctx.enter_context(tc.tile_pool(name="const", bufs=1))
        xpool = ctx.enter_context(tc.tile_pool(name="xpool", bufs=1))
        wpool = ctx.enter_context(tc.tile_pool(name="wpool", bufs=1))
        bigpool = ctx.enter_context(tc.tile_pool(name="bigpool", bufs=1))
        scratch = ctx.enter_context(tc.tile_pool(name="scratch", bufs=2))
        vecs = ctx.enter_context(tc.tile_pool(name="vecs", bufs=8))
        outp = ctx.enter_context(tc.tile_pool(name="outp", bufs=6))
        ps = ctx.enter_context(tc.tile_pool(name="ps", bufs=4, space="PSUM"))
        psg = ctx.enter_context(tc.tile_pool(name="psg", bufs=4, space="PSUM"))

        identity = const.tile([P, P], FP32)
        make_identity(nc, identity)

        # ~12 throwaway matmuls warm the PE (HAM un-throttles after ~3.4 us
        # of activity) while the first xt chunk is still in flight.
        warm = psg.tile([P, P], FP32, tag="gacc")
        for _ in range(12):
            nc.tensor.matmul(warm, identity, identity, start=True, stop=True)

        # ---- weight loads on the SWDGE path (gpsimd) so they never delay the
        # latency-critical xt stream on the Sync HWDGE queue.
        # Layout [p, cc, j]: tile[p, cc, j] = W[128*cc + p, j].
        wphiT = wpool.tile([P, CC, C], FP32R)
        wthetaT = wpool.tile([P, CC, C], FP32R)
        wg_sb = wpool.tile([P, CC, C], FP32R)
        wmT = wpool.tile([P, CC, C], FP32R)
        # Weight + x loads go on the ACT HWDGE queue (nc.scalar.dma_start)
        # so the latency-critical xt stream owns the Sync queue. Order by
        # first use: wthetaT (T1), wphiT (L), wg (A'), then x, then wmT.
        # x arrives in column-quarters: ZS pass ci only reads columns
        # [1024*ci, 1024*(ci+1)), so quarter ci unblocks that pass.
        x_sb = xpool.tile([P, CC, N], FP32R)
        QW = N // CC  # 1024

        def _load_x_quarter(ci):
            nc.scalar.dma_start(
                out=x_sb[:, :, ci * QW:(ci + 1) * QW],
                in_=x_d[:, ci * QW:(ci + 1) * QW].rearrange(
                    "(cc p) n -> p cc n", p=P
                ),
            )

        nc.scalar.dma_start(
            out=wthetaT, in_=wthetaT_d.rearrange("(cc p) j -> p cc j", p=P)
        )
        nc.scalar.dma_start(
            out=wphiT, in_=wphiT_d.rearrange("(cc p) j -> p cc j", p=P)
        )
        _load_x_quarter(0)
        nc.scalar.dma_start(
            out=wg_sb, in_=wg_d.rearrange("(cc p) j -> p cc j", p=P)
        )
        _load_x_quarter(1)
        _load_x_quarter(2)
        _load_x_quarter(3)
        nc.scalar.dma_start(
            out=wmT, in_=wmTg_d.rearrange("(cc p) j -> p cc j", p=P)
        )

        # ---- stream X^T in 1 MB chunks (4 tiles each) and fold each tile
        # into the Gram accumulators as soon as its chunk lands.
        # XT[p, t, c] = X[c, 128*t + p];  G[a, b] = sum_n X[a, n] X[b, n].
        xt_sb = bigpool.tile([P, NT, C], FP32R, tag="big")
        gacc = [
            psg.tile([P, C], FP32, tag="gacc", name=f"gacc{i}")
            for i in range(CC)
        ]
        # Ramped chunk sizes: small first chunks start the Gram stream ~2 us
        # earlier; steady-state 4-tile (1 MB) chunks keep issue overhead low.
        chunks = [2, 2, 4, 4, 4, 4, 4, 4, 4]
        t0c = 0
        for csz in chunks:
            nc.sync.dma_start(
                out=xt_sb[:, t0c:t0c + csz, :],
                in_=xt_d[t0c * P:(t0c + csz) * P, :].rearrange(
                    "(tt p) c -> p tt c", p=P
                ),
            )
            for k in range(csz):
                t = t0c + k
                for mc in range(CC):
                    nc.tensor.matmul(
                        gacc[mc],
                        xt_sb[:, t, mc * P:(mc + 1) * P],
                        xt_sb[:, t, :],
                        start=(t == 0),
                        stop=(t == NT - 1),
                    )
            t0c += csz
        assert t0c == NT

        g_sb = scratch.tile([P, CC, C], FP32R, tag="s8")
        for mc in range(CC):
            nc.any.tensor_copy(g_sb[:, mc, :], gacc[mc])

        # ---- T1 = G @ Wt^T  (uses G symmetry for the stationary operand)
        t1_sb = scratch.tile([P, CC, C], FP32R, tag="s8")
        for mc in range(CC):
            tp = ps.tile([P, C], FP32, tag="ps")
            for jc in range(CC):
                nc.tensor.matmul(
                    tp,
                    g_sb[:, jc, mc * P:(mc + 1) * P],
                    wthetaT[:, jc, :],
                    start=(jc == 0),
                    stop=(jc == CC - 1),
                )
            nc.any.tensor_copy(t1_sb[:, mc, :], tp)

        # ---- L = Wp @ T1 ; softmax rows -> attn
        attn_sb = scratch.tile([P, CC, C], FP32R, tag="s8")
        for mc in range(CC):
            lp = ps.tile([P, C], FP32, tag="ps")
            for ic in range(CC):
                nc.tensor.matmul(
                    lp,
                    wphiT[:, ic, mc * P:(mc + 1) * P],
                    t1_sb[:, ic, :],
                    start=(ic == 0),
                    stop=(ic == CC - 1),
                )
            neg_max = vecs.tile([P, 1], FP32)
            nc.vector.tensor_reduce(
                out=neg_max, in_=lp, axis=mybir.AxisListType.X,
                op=mybir.AluOpType.max, negate=True,
            )
            sums = vecs.tile([P, 1], FP32)
            nc.scalar.activation(
                out=attn_sb[:, mc, :], in_=lp,
                func=mybir.ActivationFunctionType.Exp,
                bias=neg_max, scale=1.0, accum_out=sums,
            )
            rinv = vecs.tile([P, 1], FP32)
            nc.vector.reciprocal(rinv, sums)
            nc.vector.tensor_scalar_mul(
                attn_sb[:, mc, :], attn_sb[:, mc, :], rinv
            )

        # ---- attn^T via PE transposes (fp32 mode; copies round to fp32r)
        attnT_sb = scratch.tile([P, CC, C], FP32R, tag="s8")
        for dc in range(CC):
            pt = ps.tile([P, C], FP32, tag="ps")
            for mc in range(CC):
                nc.tensor.transpose(
                    pt[:, mc * P:(mc + 1) * P],
                    _f32(attn_sb[:, mc, dc * P:(dc + 1) * P]),
                    identity,
                )
            nc.any.tensor_copy(attnT_sb[:, dc, :], pt)

        # ---- A'^T[j, c] = sum_d Wg[d, j] attn[c, d]
        apT_sb = scratch.tile([P, CC, C], FP32R, tag="s8")
        for jc in range(CC):
            ap_ps = ps.tile([P, C], FP32, tag="ps")
            for dc in range(CC):
                nc.tensor.matmul(
                    ap_ps,
                    wg_sb[:, dc, jc * P:(jc + 1) * P],
                    attnT_sb[:, dc, :],
                    start=(dc == 0),
                    stop=(dc == CC - 1),
                )
            nc.any.tensor_copy(apT_sb[:, jc, :], ap_ps)

        # ---- y^T blocks straight into Z layout, interleaved q-major with the
        # final mask GEMM + residual + store.
        # Z[i, q*512 + r] = y^T[8*i + q, r]; with n = 1024*ci + 8*m + q the
        # output PSUM tile [m, r] equals ZS[:, ci, q*512:(q+1)*512], and the
        # mask GEMM for output block jb=q only needs ZS blocks (ci=0..3, q).
        zs_sb = bigpool.tile([P, CC, N], FP32R, tag="big")
        for ci in range(CC):
            for q in range(QF):
                zp = ps.tile([P, C], FP32, tag="ps")
                for jc in range(CC):
                    xr = x_sb[:, jc, :].rearrange(
                        "p (ci m q) -> p ci q m", ci=CC, q=QF
                    )
                    nc.tensor.matmul(
                        zp,
                        xr[:, ci, q, :],
                        apT_sb[:, jc, :],
                        start=(jc == 0),
                        stop=(jc == CC - 1),
                    )
                nc.any.tensor_copy(zs_sb[:, ci, q * C:(q + 1) * C], zp)

                if ci == CC - 1:
                    # ZS blocks (0..3, q) are now all done: emit output block q
                    jb = q
                    for oc in range(CC):
                        mp = psg.tile([P, C], FP32, tag="gacc")
                        for ic in range(CC):
                            nc.tensor.matmul(
                                mp,
                                wmT[:, ic, oc * P:(oc + 1) * P],
                                zs_sb[:, ic, jb * C:(jb + 1) * C],
                                start=(ic == 0),
                                stop=(ic == CC - 1),
                            )
                        ot = outp.tile([P, C], FP32)
                        nc.vector.tensor_add(
                            ot, mp, _f32(x_sb[:, oc, jb * C:(jb + 1) * C])
                        )
                        nc.sync.dma_start(
                            out=out_d[oc * P:(oc + 1) * P, jb * C:(jb + 1) * C],
                            in_=ot,
                        )


_NC_CACHE = {}
LAST_RESULT = None


def get_nc():
    if "nc" not in _NC_CACHE:
        _NC_CACHE["nc"] = _build_nc()
    return _NC_CACHE["nc"]


def _round_fp32r(x):
    """Round fp32 array to the fp32r grid (11 explicit mantissa bits, RNE)."""
    u = np.ascontiguousarray(x, dtype=np.float32).view(np.uint32).astype(np.uint64)
    shift = 23 - 11
    add = (np.uint64(1) << np.uint64(shift - 1)) - np.uint64(1) + (
        (u >> np.uint64(shift)) & np.uint64(1)
    )
    u = (u + add) & np.uint64(~((1 << shift) - 1) & 0xFFFFFFFF)
    return u.astype(np.uint32).view(np.float32)


def make_in_map(xb, w_phi_t, w_theta_t, w_g, w_mask_t_g):
    """Per-core input dict; xb is one sample [C, H, W]."""
    xr = _round_fp32r(xb.reshape(C, N))
    return {
        "x": xr,
        "xt": np.ascontiguousarray(xr.T),
        "w_phi_t": w_phi_t,
        "w_theta_t": w_theta_t,
        "w_g": w_g,
        "w_mask_t_g": w_mask_t_g,
    }


def prep_weights(w_phi, w_theta, w_g, w_mask, gamma):
    w_phi_t = _round_fp32r(np.asarray(w_phi, dtype=np.float32).T)
    w_theta_t = _round_fp32r(np.asarray(w_theta, dtype=np.float32).T)
    w_g_r = _round_fp32r(np.asarray(w_g, dtype=np.float32))
    gamma64 = float(np.asarray(gamma, dtype=np.float32).reshape(-1)[0])
    w_mask_t_g = _round_fp32r(
        (np.asarray(w_mask, dtype=np.float64).T * gamma64).astype(np.float32)
    )
    return w_phi_t, w_theta_t, w_g_r, w_mask_t_g


def kernel(x, w_phi, w_theta, w_g, w_mask, gamma):
    global LAST_RESULT
    x = np.ascontiguousarray(np.asarray(x, dtype=np.float32))
    B, c, h, w = x.shape
    assert (c, h * w) == (C, N), (x.shape,)

    w_phi_t, w_theta_t, w_g_r, w_mask_t_g = prep_weights(
        w_phi, w_theta, w_g, w_mask, gamma
    )
    nc = get_nc()
    in_maps = [
        make_in_map(x[b], w_phi_t, w_theta_t, w_g_r, w_mask_t_g)
        for b in range(B)
    ]
    trace = bool(int(os.environ.get("KERNEL_TRACE", "0")))
    res = run_bass_kernel_spmd(nc, in_maps, list(range(B)), trace=trace)
    LAST_RESULT = res
    out = np.stack([res.results[b]["out"].reshape(c, h, w) for b in range(B)])
    return out
